# revision 11
# baseline (speedup 1.0000x reference)
# Bass/Trainium2 kernel for a double Mamba block (nn_ExBimamba).
#
# Sharding: 8 cores = 2 mamba blocks x 4 batch elements; each core runs the
# full per-(block,batch) computation with channels (d_inner) on SBUF
# partitions and time on the free axis. No collectives.
#
# Per-core pipeline:
#   P1 in_proj  : PE matmuls (K=d_model tiles), xz -> xin (SBUF, padded) + z (bf16 -> HBM scratch)
#   P2 conv1d   : PE diag-matmuls (4 taps, shifted moving operand) + ACT Silu(+bias)
#   P3 x_proj   : PE matmuls -> (dt|B|C); B,C broadcast to 128 partitions via HBM-bounce DMA
#   P4 scan     : per 128-ch tile g, per state n:
#                   a = ACT Exp(A[:,n] * softplus(dt_proj))   (per-partition scale)
#                   w = du16 * B_bc[n]                        (GPSIMD, bf16)
#                   h = tensor_tensor_scan(a, w)              (DVE recurrence)
#                   X = h * C_bc[n]                           (GPSIMD, bf16)
#                   y += I.T @ X                              (PE PSUM accumulate over n)
#                 then y2 = u*D + y ; y3 = y2 * silu(z)
#   P5 out_proj : PE matmuls (bf16) -> out (d_model x L), DMA out
import numpy as np
import ml_dtypes

import bass_rust
import concourse.bass as bass
import concourse.mybir as mybir
import concourse.tile as tile
from concourse.bass_utils import run_bass_kernel_spmd

F32 = mybir.dt.float32
BF16 = mybir.dt.bfloat16
AF = mybir.ActivationFunctionType
OP = mybir.AluOpType


def _split_waits(nc, max_waits=1):
    # The walrus build in this container rejects >1 sync-wait per
    # instruction; hoist extras onto preceding same-engine NoOps.
    for f in nc.m.functions:
        for bb in f.blocks:
            out = []
            for inst in bb.instructions:
                si = inst.sync_info
                if si is not None and len(si.on_wait) > max_waits:
                    waits = list(si.on_wait)
                    keep = waits[-max_waits:]
                    rest = waits[:-max_waits]
                    for i in range(0, len(rest), max_waits):
                        nop = mybir.InstNoOp(name=f"{inst.name}_ws{i}")
                        nop.engine = inst.engine
                        nop.sync_info = bass_rust.SyncInfo(
                            on_wait=rest[i : i + max_waits], on_update=[]
                        )
                        out.append(nop)
                    si.on_wait = keep
                out.append(inst)
            bb.instructions[:] = out


def build_nc(L=1024, DM=1024, DI=2048, N=16, R=64, num_devices=8, split_waits=True):
    """Build the per-core Bass program (SPMD: same program, per-core data)."""
    G = DI // 128      # d_inner tiles
    DMT = DM // 128    # d_model tiles (contraction for in_proj)
    E2 = 2 * DI // 128 # in_proj output tiles
    ET = DM // 128     # out_proj output tiles
    KH = 512           # fp32 moving free-dim max
    NH = L // KH if L >= KH else 1
    KHL = min(KH, L)

    nc = bass.Bass("TRN2", target_bir_lowering=False, debug=False,
                   num_devices=num_devices)

    # ---- external I/O (per core) ----
    xT = nc.declare_dram_parameter("xT", [DM, L], F32, isOutput=False)
    wipT = nc.declare_dram_parameter("wipT", [DM, 2 * DI], F32, isOutput=False)
    convw = nc.declare_dram_parameter("convw", [DI, 4], F32, isOutput=False)
    convb = nc.declare_dram_parameter("convb", [DI, 1], F32, isOutput=False)
    wxT = nc.declare_dram_parameter("wxT", [DI, R + 2 * N], BF16, isOutput=False)
    wdtT = nc.declare_dram_parameter("wdtT", [R, DI], F32, isOutput=False)
    dtb = nc.declare_dram_parameter("dtb", [DI, 1], F32, isOutput=False)
    acol = nc.declare_dram_parameter("acol", [DI, N], F32, isOutput=False)
    dcol = nc.declare_dram_parameter("dcol", [DI, 1], F32, isOutput=False)
    woutT = nc.declare_dram_parameter("woutT", [DI, DM], BF16, isOutput=False)
    eye32 = nc.declare_dram_parameter("eye32", [128, 128], F32, isOutput=False)
    eyebf = nc.declare_dram_parameter("eyebf", [128, 128], BF16, isOutput=False)
    outT = nc.declare_dram_parameter("outT", [DM, L], F32, isOutput=True)

    # ---- DRAM scratch ----
    z16_hbm = nc.dram_tensor("z16_scratch", [DI, L], BF16)
    bc_hbm = nc.dram_tensor("bc_scratch", [2 * N, L], BF16)

    from contextlib import ExitStack
    with tile.TileContext(nc) as tc:
        # persistent pools
        es0 = ExitStack()
        singles = es0.enter_context(tc.tile_pool(name="singles", bufs=1))
        u16_pool = es0.enter_context(tc.tile_pool(name="u16", bufs=1))
        bcst = es0.enter_context(tc.tile_pool(name="bcst", bufs=1))
        y3_pool = es0.enter_context(tc.tile_pool(name="y3", bufs=1))

        convw_sb = singles.tile([128, G, 4], F32)
        nc.sync.dma_start(convw_sb, convw.ap().rearrange("(g p) k -> p g k", p=128))
        convb_sb = singles.tile([128, G], F32)
        nc.sync.dma_start(convb_sb, convb.ap().rearrange("(g p) k -> p (g k)", p=128))
        dtb_sb = singles.tile([128, G], F32)
        nc.sync.dma_start(dtb_sb, dtb.ap().rearrange("(g p) k -> p (g k)", p=128))
        dcol_sb = singles.tile([128, G], F32)
        nc.sync.dma_start(dcol_sb, dcol.ap().rearrange("(g p) k -> p (g k)", p=128))
        acol_sb = singles.tile([128, G, N], F32)
        nc.sync.dma_start(acol_sb, acol.ap().rearrange("(g p) n -> p g n", p=128))
        eye32_sb = singles.tile([128, 128], F32)
        nc.sync.dma_start(eye32_sb, eye32.ap())
        eyebf_sb = singles.tile([128, 128], BF16)
        nc.sync.dma_start(eyebf_sb, eyebf.ap())

        u16_t = [u16_pool.tile([128, L], BF16, name=f"u16_{i}", tag=f"u16_{i}") for i in range(G)]
        y3_t = [y3_pool.tile([128, L], BF16, name=f"y3_{i}", tag=f"y3_{i}") for i in range(G)]

        # ---------------- P1: in_proj + P2: conv ----------------
        es1 = ExitStack()   # pools alive through P4
        esA = ExitStack()   # P1/P2-only pools
        xc_pool = esA.enter_context(tc.tile_pool(name="xc", bufs=2))
        xt_pool = esA.enter_context(tc.tile_pool(name="xt", bufs=DMT))
        wip_pool = esA.enter_context(tc.tile_pool(name="wip", bufs=4))
        xin_pool = esA.enter_context(tc.tile_pool(name="xin", bufs=3))
        zst_pool = esA.enter_context(tc.tile_pool(name="zst", bufs=3))
        diag_pool = esA.enter_context(tc.tile_pool(name="diag", bufs=8))
        p_xz = esA.enter_context(tc.tile_pool(name="p_xz", bufs=2, space="PSUM"))
        p_up = esA.enter_context(tc.tile_pool(name="p_up", bufs=2, space="PSUM"))
        if True:

            xt_t = []
            for dm in range(DMT):
                t = xt_pool.tile([128, L], F32)
                nc.sync.dma_start(t, xT.ap()[dm * 128:(dm + 1) * 128, :])
                xt_t.append(t)

            xin_t = []
            for e in range(E2):
                ps = p_xz.tile([128, L], F32)
                for dm in range(DMT):
                    wt = wip_pool.tile([128, 128], F32)
                    nc.sync.dma_start(
                        wt, wipT.ap()[dm * 128:(dm + 1) * 128,
                                      e * 128:(e + 1) * 128])
                    for h in range(NH):
                        nc.tensor.matmul(
                            ps[:, h * KHL:(h + 1) * KHL], wt,
                            xt_t[dm][:, h * KHL:(h + 1) * KHL],
                            start=(dm == 0), stop=(dm == DMT - 1))
                if e < G:
                    xi = xin_pool.tile([128, L + 4], F32)
                    nc.vector.memset(xi[:, 0:4], 0.0)
                    nc.scalar.copy(xi[:, 4:4 + L], ps)
                    xin_t.append(xi)
                    # conv for this tile (xin slot freed right after)
                    g = e
                    up = p_up.tile([128, L], F32)
                    for k in range(4):
                        dg = diag_pool.tile([128, 128], F32)
                        nc.vector.tensor_scalar_mul(
                            dg, eye32_sb, convw_sb[:, g, k:k + 1])
                        for h in range(NH):
                            nc.tensor.matmul(
                                up[:, h * KHL:(h + 1) * KHL], dg,
                                xi[:, 1 + k + h * KHL:1 + k + h * KHL + KHL],
                                start=(k == 0), stop=(k == 3))
                    xc = xc_pool.tile([128, L], F32, name=f"xc_{e}", tag="xc")
                    nc.scalar.activation(xc, up, AF.Identity,
                                         bias=convb_sb[:, g:g + 1], scale=1.0)
                    sg = xc_pool.tile([128, L], F32, name=f"sg_{e}", tag="sg")
                    nc.scalar.activation(sg, up, AF.Sigmoid,
                                         bias=convb_sb[:, g:g + 1], scale=1.0)
                    nc.vector.tensor_mul(u16_t[g], xc, sg)
                else:
                    zt = zst_pool.tile([128, L], BF16)
                    nc.scalar.copy(zt, ps)
                    r = e - G
                    nc.sync.dma_start(
                        z16_hbm.ap()[r * 128:(r + 1) * 128, :], zt)

            # ---------------- P3: x_proj ----------------
            esA.close()
            wx_pool = es1.enter_context(tc.tile_pool(name="wx", bufs=2))
            xdbl_pool = es1.enter_context(tc.tile_pool(name="xdbl", bufs=1))
            bc16_pool = es1.enter_context(tc.tile_pool(name="bc16", bufs=1))
            esB = ExitStack()
            p_xd = esB.enter_context(tc.tile_pool(name="p_xd", bufs=1, space="PSUM"))
            if True:
                F = R + 2 * N
                xd = p_xd.tile([F, L], F32)
                for g in range(G):
                    wx = wx_pool.tile([128, F], BF16)
                    nc.sync.dma_start(wx, wxT.ap()[g * 128:(g + 1) * 128, :])
                    for h in range(NH):
                        nc.tensor.matmul(
                            xd[:, h * KHL:(h + 1) * KHL], wx,
                            u16_t[g][:, h * KHL:(h + 1) * KHL],
                            start=(g == 0), stop=(g == G - 1))
                xdbl_sb = xdbl_pool.tile([F, L], F32)
                nc.scalar.copy(xdbl_sb, xd)
                bc16 = bc16_pool.tile([2 * N, L], BF16)
                nc.vector.tensor_copy(bc16, xdbl_sb[R:R + 2 * N, :])
                nc.sync.dma_start(bc_hbm.ap(), bc16)

                b_bc = []
                c_bc = []
                for n in range(N):
                    bt = bcst.tile([128, L], BF16, name=f"bbc_{n}", tag=f"bbc_{n}")
                    nc.sync.dma_start(
                        bt, bc_hbm.ap()[n:n + 1, :].to_broadcast((128, L)))
                    b_bc.append(bt)
                for n in range(N):
                    ct = bcst.tile([128, L], BF16, name=f"cbc_{n}", tag=f"cbc_{n}")
                    nc.sync.dma_start(
                        ct, bc_hbm.ap()[N + n:N + n + 1, :].to_broadcast((128, L)))
                    c_bc.append(ct)

                # ---------------- P4: dt_proj + scan ----------------
                esB.close()
                wdt_pool = es1.enter_context(tc.tile_pool(name="wdt", bufs=2))
                d_pool = es1.enter_context(tc.tile_pool(name="delta", bufs=2))
                du_pool = es1.enter_context(tc.tile_pool(name="du16", bufs=2))
                w_pool = es1.enter_context(tc.tile_pool(name="w2", bufs=3))
                h_pool = es1.enter_context(tc.tile_pool(name="h2", bufs=3))
                x_pool = es1.enter_context(tc.tile_pool(name="X2", bufs=3))
                zin_pool = es1.enter_context(tc.tile_pool(name="zin", bufs=2))
                sz_pool = es1.enter_context(tc.tile_pool(name="sz", bufs=2))
                t1_pool = es1.enter_context(tc.tile_pool(name="t1", bufs=2))
                y2_pool = es1.enter_context(tc.tile_pool(name="y2", bufs=2))
                p_a = es1.enter_context(tc.tile_pool(name="p_a", bufs=3, space="PSUM"))
                p_y = es1.enter_context(tc.tile_pool(name="p_y", bufs=1, space="PSUM"))
                if True:
                    for g in range(G):
                        dtp = p_a.tile([128, L], F32, name=f"aps_dt{g}", tag="a_ps")
                        wdt = wdt_pool.tile([R, 128], F32)
                        nc.sync.dma_start(
                            wdt, wdtT.ap()[:, g * 128:(g + 1) * 128])
                        for h in range(NH):
                            nc.tensor.matmul(
                                dtp[:, h * KHL:(h + 1) * KHL], wdt,
                                xdbl_sb[0:R, h * KHL:(h + 1) * KHL],
                                start=True, stop=True)
                        edt = d_pool.tile([128, L], F32, name=f"edt_{g}", tag="edt")
                        nc.scalar.activation(edt, dtp, AF.Exp,
                                             bias=dtb_sb[:, g:g + 1], scale=1.0)
                        delta = d_pool.tile([128, L], F32, name=f"delta_{g}", tag="delta")
                        nc.scalar.activation(delta, edt, AF.Ln, bias=1.0, scale=1.0)
                        du16 = du_pool.tile([128, L], BF16)
                        nc.vector.tensor_mul(du16, delta, u16_t[g])

                        y_ps = p_y.tile([128, L], F32)
                        for n in range(N):
                            a = p_a.tile([128, L], F32, name=f"aps_{g}_{n}", tag="a_ps")
                            nc.scalar.activation(a, delta, AF.Exp,
                                                 scale=acol_sb[:, g, n:n + 1])
                            w2 = w_pool.tile([128, L], BF16)
                            nc.gpsimd.tensor_mul(w2, du16, b_bc[n])
                            h2 = h_pool.tile([128, L], BF16)
                            nc.vector.tensor_tensor_scan(
                                h2, a, w2, 0.0, op0=OP.mult, op1=OP.add)
                            X2 = x_pool.tile([128, L], BF16)
                            nc.gpsimd.tensor_mul(X2, h2, c_bc[n])
                            for h in range(NH):
                                nc.tensor.matmul(
                                    y_ps[:, h * KHL:(h + 1) * KHL], eyebf_sb,
                                    X2[:, h * KHL:(h + 1) * KHL],
                                    start=(n == 0), stop=(n == N - 1))
                        t1 = t1_pool.tile([128, L], F32)
                        nc.gpsimd.tensor_scalar_mul(t1, u16_t[g],
                                                    dcol_sb[:, g:g + 1])
                        y2 = y2_pool.tile([128, L], F32)
                        nc.vector.tensor_add(y2, t1, y_ps)
                        zt = zin_pool.tile([128, L], BF16)
                        nc.sync.dma_start(
                            zt, z16_hbm.ap()[g * 128:(g + 1) * 128, :])
                        sz = sz_pool.tile([128, L], BF16)
                        nc.scalar.activation(sz, zt, AF.Sigmoid)
                        y3a = sz_pool.tile([128, L], F32, name=f"y3a_{g}", tag="y3a")
                        nc.gpsimd.tensor_mul(y3a, y2, zt)
                        nc.gpsimd.tensor_mul(y3_t[g], y3a, sz)

        # ---------------- P5: out_proj ----------------
        es1.close()
        es5 = ExitStack()
        wo_pool = es5.enter_context(tc.tile_pool(name="wo", bufs=4))
        osb_pool = es5.enter_context(tc.tile_pool(name="osb", bufs=3))
        p_out = es5.enter_context(tc.tile_pool(name="p_out", bufs=3, space="PSUM"))
        if True:
            for e in range(ET):
                ps = p_out.tile([128, L], F32)
                for g in range(G):
                    wo = wo_pool.tile([128, 128], BF16)
                    nc.sync.dma_start(
                        wo, woutT.ap()[g * 128:(g + 1) * 128,
                                       e * 128:(e + 1) * 128])
                    for h in range(NH):
                        nc.tensor.matmul(
                            ps[:, h * KHL:(h + 1) * KHL], wo,
                            y3_t[g][:, h * KHL:(h + 1) * KHL],
                            start=(g == 0), stop=(g == G - 1))
                osb = osb_pool.tile([128, L], F32)
                nc.scalar.copy(osb, ps)
                nc.sync.dma_start(outT.ap()[e * 128:(e + 1) * 128, :], osb)

        es5.close()
        es0.close()

    if split_waits:
        _split_waits(nc)
    return nc


def _prep_core_inputs(x_b, p, L, DM, DI, N, R):
    """Host-side packing for one core. p = tuple of 9 block params."""
    (in_proj_w, conv_w, conv_b, x_proj_w, dt_proj_w, dt_proj_b,
     A_log, D_param, out_proj_w) = p
    bf = ml_dtypes.bfloat16
    f32 = np.float32
    return {
        "xT": np.ascontiguousarray(x_b.T, dtype=f32),
        "wipT": np.ascontiguousarray(in_proj_w.T, dtype=f32),
        "convw": np.ascontiguousarray(conv_w, dtype=f32),
        "convb": np.ascontiguousarray(conv_b.reshape(DI, 1), dtype=f32),
        "wxT": np.ascontiguousarray(x_proj_w.T.astype(np.float32)).astype(bf),
        "wdtT": np.ascontiguousarray(dt_proj_w.T, dtype=f32),
        "dtb": np.ascontiguousarray(dt_proj_b.reshape(DI, 1), dtype=f32),
        "acol": np.ascontiguousarray(-np.exp(A_log), dtype=f32),
        "dcol": np.ascontiguousarray(D_param.reshape(DI, 1), dtype=f32),
        "woutT": np.ascontiguousarray(out_proj_w.T).astype(bf),
        "eye32": np.eye(128, dtype=f32),
        "eyebf": np.eye(128).astype(bf),
    }


_NC_CACHE = {}


def _get_nc():
    if "nc" not in _NC_CACHE:
        _NC_CACHE["nc"] = build_nc()
    return _NC_CACHE["nc"]


_PNAMES = ["in_proj_w", "conv_w", "conv_b", "x_proj_w", "dt_proj_w",
           "dt_proj_b", "A_log", "D_param", "out_proj_w"]


def kernel(**inputs):
    L, DM, DI, N, R = 1024, 1024, 2048, 16, 64
    hidden = inputs["hidden"]
    diff = inputs["diff"]
    hp = tuple(np.asarray(inputs["h_" + n]) for n in _PNAMES)
    dp = tuple(np.asarray(inputs["d_" + n]) for n in _PNAMES)

    nc = _get_nc()
    in_maps = []
    for c in range(8):
        blk, x, p = (("h", hidden, hp) if c < 4 else ("d", diff, dp))
        b = c % 4
        in_maps.append(_prep_core_inputs(np.asarray(x[b]), p, L, DM, DI, N, R))
    res = run_bass_kernel_spmd(nc, in_maps, core_ids=list(range(8)))
    outs = [np.ascontiguousarray(res.results[c]["outT"].T) for c in range(8)]
    hidden_out = np.stack(outs[0:4], axis=0).astype(np.float32)
    diff_out = np.stack(outs[4:8], axis=0).astype(np.float32)
    return (hidden_out, diff_out)


# revision 16
# speedup vs baseline: 1.2865x; 1.2865x over previous
# Bass/Trainium2 kernel for a double Mamba block (nn_ExBimamba).
#
# Sharding: 8 cores = 2 mamba blocks x 4 batch elements; each core runs the
# full per-(block,batch) computation with channels (d_inner) on SBUF
# partitions and time on the free axis. No collectives.
#
# Per-core pipeline:
#   P1 in_proj  : PE matmuls (K=d_model tiles), xz -> xin (SBUF, padded) + z (bf16 -> HBM scratch)
#   P2 conv1d   : PE diag-matmuls (4 taps, shifted moving operand) + ACT Silu(+bias)
#   P3 x_proj   : PE matmuls -> (dt|B|C); B,C broadcast to 128 partitions via HBM-bounce DMA
#   P4 scan     : per 128-ch tile g, per state n:
#                   a = ACT Exp(A[:,n] * softplus(dt_proj))   (per-partition scale)
#                   w = du16 * B_bc[n]                        (GPSIMD, bf16)
#                   h = tensor_tensor_scan(a, w)              (DVE recurrence)
#                   X = h * C_bc[n]                           (GPSIMD, bf16)
#                   y += I.T @ X                              (PE PSUM accumulate over n)
#                 then y2 = u*D + y ; y3 = y2 * silu(z)
#   P5 out_proj : PE matmuls (bf16) -> out (d_model x L), DMA out
import numpy as np
import ml_dtypes

import bass_rust
import concourse.bass as bass
import concourse.mybir as mybir
import concourse.tile as tile
from concourse.bass_utils import run_bass_kernel_spmd

F32 = mybir.dt.float32
BF16 = mybir.dt.bfloat16
AF = mybir.ActivationFunctionType
OP = mybir.AluOpType


def _split_waits(nc, max_waits=1):
    # The walrus build in this container rejects >1 sync-wait per
    # instruction; hoist extras onto preceding same-engine NoOps.
    for f in nc.m.functions:
        for bb in f.blocks:
            out = []
            for inst in bb.instructions:
                si = inst.sync_info
                if si is not None and len(si.on_wait) > max_waits:
                    waits = list(si.on_wait)
                    keep = waits[-max_waits:]
                    rest = waits[:-max_waits]
                    for i in range(0, len(rest), max_waits):
                        nop = mybir.InstNoOp(name=f"{inst.name}_ws{i}")
                        nop.engine = inst.engine
                        nop.sync_info = bass_rust.SyncInfo(
                            on_wait=rest[i : i + max_waits], on_update=[]
                        )
                        out.append(nop)
                    si.on_wait = keep
                out.append(inst)
            bb.instructions[:] = out


def build_nc(L=1024, DM=1024, DI=2048, N=16, R=64, num_devices=8, split_waits=True):
    """Build the per-core Bass program (SPMD: same program, per-core data)."""
    G = DI // 128      # d_inner tiles
    DMT = DM // 128    # d_model tiles (contraction for in_proj)
    E2 = 2 * DI // 128 # in_proj output tiles
    ET = DM // 128     # out_proj output tiles
    KH = 512           # fp32 moving free-dim max
    NH = L // KH if L >= KH else 1
    KHL = min(KH, L)

    nc = bass.Bass("TRN2", target_bir_lowering=False, debug=False,
                   num_devices=num_devices)

    # ---- external I/O (per core) ----
    xT = nc.declare_dram_parameter("xT", [DM, L], F32, isOutput=False)
    wipT = nc.declare_dram_parameter("wipT", [DM, 2 * DI], F32, isOutput=False)
    convw = nc.declare_dram_parameter("convw", [DI, 4], F32, isOutput=False)
    convb = nc.declare_dram_parameter("convb", [DI, 1], F32, isOutput=False)
    wxT = nc.declare_dram_parameter("wxT", [DI, R + 2 * N], BF16, isOutput=False)
    wdtT = nc.declare_dram_parameter("wdtT", [R, DI], F32, isOutput=False)
    dtb = nc.declare_dram_parameter("dtb", [DI, 1], F32, isOutput=False)
    acol = nc.declare_dram_parameter("acol", [DI, N], F32, isOutput=False)
    dcol = nc.declare_dram_parameter("dcol", [DI, 1], F32, isOutput=False)
    woutT = nc.declare_dram_parameter("woutT", [DI, DM], BF16, isOutput=False)
    eye32 = nc.declare_dram_parameter("eye32", [128, 128], F32, isOutput=False)
    eyebf = nc.declare_dram_parameter("eyebf", [128, 128], BF16, isOutput=False)
    outT = nc.declare_dram_parameter("outT", [DM, L], F32, isOutput=True)

    # ---- DRAM scratch ----
    z16_hbm = nc.dram_tensor("z16_scratch", [DI, L], BF16)
    bc_hbm = nc.dram_tensor("bc_scratch", [2 * N, L], BF16)

    from contextlib import ExitStack
    with tile.TileContext(nc) as tc:
        # persistent pools
        es0 = ExitStack()
        singles = es0.enter_context(tc.tile_pool(name="singles", bufs=1))
        u16_pool = es0.enter_context(tc.tile_pool(name="u16", bufs=1))
        bcst = es0.enter_context(tc.tile_pool(name="bcst", bufs=1))
        y3_pool = es0.enter_context(tc.tile_pool(name="y3", bufs=1))

        convw_sb = singles.tile([128, G, 4], F32)
        nc.sync.dma_start(convw_sb, convw.ap().rearrange("(g p) k -> p g k", p=128))
        convb_sb = singles.tile([128, G], F32)
        nc.sync.dma_start(convb_sb, convb.ap().rearrange("(g p) k -> p (g k)", p=128))
        dtb_sb = singles.tile([128, G], F32)
        nc.sync.dma_start(dtb_sb, dtb.ap().rearrange("(g p) k -> p (g k)", p=128))
        dcol_sb = singles.tile([128, G], F32)
        nc.sync.dma_start(dcol_sb, dcol.ap().rearrange("(g p) k -> p (g k)", p=128))
        acol_sb = singles.tile([128, G, N], F32)
        nc.sync.dma_start(acol_sb, acol.ap().rearrange("(g p) n -> p g n", p=128))
        eye32_sb = singles.tile([128, 128], F32)
        nc.sync.dma_start(eye32_sb, eye32.ap())
        eyebf_sb = singles.tile([128, 128], BF16)
        nc.sync.dma_start(eyebf_sb, eyebf.ap())

        u16_t = [u16_pool.tile([128, L], BF16, name=f"u16_{i}", tag=f"u16_{i}") for i in range(G)]
        y3_t = [y3_pool.tile([128, L], BF16, name=f"y3_{i}", tag=f"y3_{i}") for i in range(G)]

        # ---------------- P1: in_proj + P2: conv ----------------
        es1 = ExitStack()   # pools alive through P4
        esA = ExitStack()   # P1/P2-only pools
        xc_pool = esA.enter_context(tc.tile_pool(name="xc", bufs=2))
        xt_pool = esA.enter_context(tc.tile_pool(name="xt", bufs=DMT))
        wip_pool = esA.enter_context(tc.tile_pool(name="wip", bufs=4))
        xin_pool = esA.enter_context(tc.tile_pool(name="xin", bufs=3))
        zst_pool = esA.enter_context(tc.tile_pool(name="zst", bufs=3))
        diag_pool = esA.enter_context(tc.tile_pool(name="diag", bufs=8))
        p_xz = esA.enter_context(tc.tile_pool(name="p_xz", bufs=2, space="PSUM"))
        p_up = esA.enter_context(tc.tile_pool(name="p_up", bufs=2, space="PSUM"))
        if True:

            xt_t = []
            for dm in range(DMT):
                t = xt_pool.tile([128, L], F32)
                nc.sync.dma_start(t, xT.ap()[dm * 128:(dm + 1) * 128, :])
                xt_t.append(t)

            xin_t = []
            for e in range(E2):
                ps = p_xz.tile([128, L], F32)
                for dm in range(DMT):
                    wt = wip_pool.tile([128, 128], F32)
                    nc.sync.dma_start(
                        wt, wipT.ap()[dm * 128:(dm + 1) * 128,
                                      e * 128:(e + 1) * 128])
                    for h in range(NH):
                        nc.tensor.matmul(
                            ps[:, h * KHL:(h + 1) * KHL], wt,
                            xt_t[dm][:, h * KHL:(h + 1) * KHL],
                            start=(dm == 0), stop=(dm == DMT - 1))
                if e < G:
                    xi = xin_pool.tile([128, L + 4], F32)
                    nc.vector.memset(xi[:, 0:4], 0.0)
                    nc.scalar.copy(xi[:, 4:4 + L], ps)
                    xin_t.append(xi)
                    # conv for this tile (xin slot freed right after)
                    g = e
                    up = p_up.tile([128, L], F32)
                    for k in range(4):
                        dg = diag_pool.tile([128, 128], F32)
                        nc.vector.tensor_scalar_mul(
                            dg, eye32_sb, convw_sb[:, g, k:k + 1])
                        for h in range(NH):
                            nc.tensor.matmul(
                                up[:, h * KHL:(h + 1) * KHL], dg,
                                xi[:, 1 + k + h * KHL:1 + k + h * KHL + KHL],
                                start=(k == 0), stop=(k == 3))
                    xc = xc_pool.tile([128, L], F32, name=f"xc_{e}", tag="xc")
                    nc.scalar.activation(xc, up, AF.Identity,
                                         bias=convb_sb[:, g:g + 1], scale=1.0)
                    sg = xc_pool.tile([128, L], F32, name=f"sg_{e}", tag="sg")
                    nc.scalar.activation(sg, up, AF.Sigmoid,
                                         bias=convb_sb[:, g:g + 1], scale=1.0)
                    nc.vector.tensor_mul(u16_t[g], xc, sg)
                else:
                    zt = zst_pool.tile([128, L], BF16)
                    nc.scalar.copy(zt, ps)
                    r = e - G
                    nc.sync.dma_start(
                        z16_hbm.ap()[r * 128:(r + 1) * 128, :], zt)

            # ---------------- P3: x_proj ----------------
            esA.close()
            wx_pool = es1.enter_context(tc.tile_pool(name="wx", bufs=2))
            xdbl_pool = es1.enter_context(tc.tile_pool(name="xdbl", bufs=1))
            bc16_pool = es1.enter_context(tc.tile_pool(name="bc16", bufs=1))
            esB = ExitStack()
            p_xd = esB.enter_context(tc.tile_pool(name="p_xd", bufs=1, space="PSUM"))
            if True:
                F = R + 2 * N
                xd = p_xd.tile([F, L], F32)
                for g in range(G):
                    wx = wx_pool.tile([128, F], BF16)
                    nc.sync.dma_start(wx, wxT.ap()[g * 128:(g + 1) * 128, :])
                    for h in range(NH):
                        nc.tensor.matmul(
                            xd[:, h * KHL:(h + 1) * KHL], wx,
                            u16_t[g][:, h * KHL:(h + 1) * KHL],
                            start=(g == 0), stop=(g == G - 1))
                xdbl_sb = xdbl_pool.tile([F, L], F32)
                nc.scalar.copy(xdbl_sb, xd)
                bc16 = bc16_pool.tile([2 * N, L], BF16)
                nc.vector.tensor_copy(bc16, xdbl_sb[R:R + 2 * N, :])
                nc.sync.dma_start(bc_hbm.ap(), bc16)

                b_bc = []
                c_bc = []
                for n in range(N):
                    bt = bcst.tile([128, L], BF16, name=f"bbc_{n}", tag=f"bbc_{n}")
                    nc.sync.dma_start(
                        bt, bc_hbm.ap()[n:n + 1, :].to_broadcast((128, L)))
                    b_bc.append(bt)
                for n in range(N):
                    ct = bcst.tile([128, L], BF16, name=f"cbc_{n}", tag=f"cbc_{n}")
                    nc.sync.dma_start(
                        ct, bc_hbm.ap()[N + n:N + n + 1, :].to_broadcast((128, L)))
                    c_bc.append(ct)

                # ---------------- P4: dt_proj + scan ----------------
                esB.close()
                wdt_pool = es1.enter_context(tc.tile_pool(name="wdt", bufs=2))
                a_pool = es1.enter_context(tc.tile_pool(name="a_sb", bufs=3))
                d_pool = es1.enter_context(tc.tile_pool(name="delta", bufs=2))
                du_pool = es1.enter_context(tc.tile_pool(name="du16", bufs=2))
                w_pool = es1.enter_context(tc.tile_pool(name="w2", bufs=4))
                h_pool = es1.enter_context(tc.tile_pool(name="h2", bufs=4))
                x_pool = es1.enter_context(tc.tile_pool(name="X2", bufs=3))
                zin_pool = es1.enter_context(tc.tile_pool(name="zin", bufs=2))
                sz_pool = es1.enter_context(tc.tile_pool(name="sz", bufs=2))
                t1_pool = es1.enter_context(tc.tile_pool(name="t1", bufs=2))
                y2_pool = es1.enter_context(tc.tile_pool(name="y2", bufs=2))
                p_a = es1.enter_context(tc.tile_pool(name="p_a", bufs=2, space="PSUM"))
                p_y = es1.enter_context(tc.tile_pool(name="p_y", bufs=2, space="PSUM"))
                if True:
                    for g in range(G):
                        dtp = p_a.tile([128, L], F32, name=f"dtp_{g}", tag="dt_ps")
                        wdt = wdt_pool.tile([R, 128], F32)
                        nc.sync.dma_start(
                            wdt, wdtT.ap()[:, g * 128:(g + 1) * 128])
                        for h in range(NH):
                            nc.tensor.matmul(
                                dtp[:, h * KHL:(h + 1) * KHL], wdt,
                                xdbl_sb[0:R, h * KHL:(h + 1) * KHL],
                                start=True, stop=True)
                        edt = d_pool.tile([128, L], F32, name=f"edt_{g}", tag="edt")
                        nc.scalar.activation(edt, dtp, AF.Exp,
                                             bias=dtb_sb[:, g:g + 1], scale=1.0)
                        delta = d_pool.tile([128, L], F32, name=f"delta_{g}", tag="delta")
                        nc.scalar.activation(delta, edt, AF.Ln, bias=1.0, scale=1.0)
                        du16 = du_pool.tile([128, L], BF16)
                        nc.vector.tensor_mul(du16, delta, u16_t[g])

                        y_ps = p_y.tile([128, L], F32)
                        for n in range(N):
                            a = a_pool.tile([128, L], BF16, name=f"a_{g}_{n}", tag="a_sb")
                            nc.scalar.activation(a, delta, AF.Exp,
                                                 scale=acol_sb[:, g, n:n + 1])
                            w2 = w_pool.tile([128, L], BF16)
                            weng = nc.gpsimd if (n % 2 == 0) else nc.vector
                            weng.tensor_mul(w2, du16, b_bc[n])
                            h2 = h_pool.tile([128, L], BF16)
                            nc.vector.tensor_tensor_scan(
                                h2, a, w2, 0.0, op0=OP.mult, op1=OP.add)
                            X2 = x_pool.tile([128, L], BF16)
                            nc.vector.tensor_mul(X2, h2, c_bc[n])
                            for h in range(NH):
                                nc.tensor.matmul(
                                    y_ps[:, h * KHL:(h + 1) * KHL], eyebf_sb,
                                    X2[:, h * KHL:(h + 1) * KHL],
                                    start=(n == 0), stop=(n == N - 1))
                        t1 = t1_pool.tile([128, L], BF16)
                        nc.vector.tensor_scalar_mul(t1, u16_t[g],
                                                    dcol_sb[:, g:g + 1])
                        y2 = y2_pool.tile([128, L], BF16)
                        nc.vector.tensor_add(y2, t1, y_ps)
                        zt = zin_pool.tile([128, L], BF16)
                        nc.sync.dma_start(
                            zt, z16_hbm.ap()[g * 128:(g + 1) * 128, :])
                        sz = sz_pool.tile([128, L], BF16)
                        nc.scalar.activation(sz, zt, AF.Sigmoid)
                        y3a = sz_pool.tile([128, L], BF16, name=f"y3a_{g}", tag="y3a")
                        nc.gpsimd.tensor_mul(y3a, y2, zt)
                        nc.vector.tensor_mul(y3_t[g], y3a, sz)

        # ---------------- P5: out_proj ----------------
        es1.close()
        es5 = ExitStack()
        wo_pool = es5.enter_context(tc.tile_pool(name="wo", bufs=4))
        osb_pool = es5.enter_context(tc.tile_pool(name="osb", bufs=3))
        p_out = es5.enter_context(tc.tile_pool(name="p_out", bufs=3, space="PSUM"))
        if True:
            for e in range(ET):
                ps = p_out.tile([128, L], F32)
                for g in range(G):
                    wo = wo_pool.tile([128, 128], BF16)
                    nc.sync.dma_start(
                        wo, woutT.ap()[g * 128:(g + 1) * 128,
                                       e * 128:(e + 1) * 128])
                    for h in range(NH):
                        nc.tensor.matmul(
                            ps[:, h * KHL:(h + 1) * KHL], wo,
                            y3_t[g][:, h * KHL:(h + 1) * KHL],
                            start=(g == 0), stop=(g == G - 1))
                osb = osb_pool.tile([128, L], F32)
                nc.scalar.copy(osb, ps)
                nc.sync.dma_start(outT.ap()[e * 128:(e + 1) * 128, :], osb)

        es5.close()
        es0.close()

    if split_waits:
        _split_waits(nc)
    return nc


def _prep_core_inputs(x_b, p, L, DM, DI, N, R):
    """Host-side packing for one core. p = tuple of 9 block params."""
    (in_proj_w, conv_w, conv_b, x_proj_w, dt_proj_w, dt_proj_b,
     A_log, D_param, out_proj_w) = p
    bf = ml_dtypes.bfloat16
    f32 = np.float32
    return {
        "xT": np.ascontiguousarray(x_b.T, dtype=f32),
        "wipT": np.ascontiguousarray(in_proj_w.T, dtype=f32),
        "convw": np.ascontiguousarray(conv_w, dtype=f32),
        "convb": np.ascontiguousarray(conv_b.reshape(DI, 1), dtype=f32),
        "wxT": np.ascontiguousarray(x_proj_w.T.astype(np.float32)).astype(bf),
        "wdtT": np.ascontiguousarray(dt_proj_w.T, dtype=f32),
        "dtb": np.ascontiguousarray(dt_proj_b.reshape(DI, 1), dtype=f32),
        "acol": np.ascontiguousarray(-np.exp(A_log), dtype=f32),
        "dcol": np.ascontiguousarray(D_param.reshape(DI, 1), dtype=f32),
        "woutT": np.ascontiguousarray(out_proj_w.T).astype(bf),
        "eye32": np.eye(128, dtype=f32),
        "eyebf": np.eye(128).astype(bf),
    }


LAST_RUN_SECONDS = None
_NC_CACHE = {}


def _get_nc():
    if "nc" not in _NC_CACHE:
        _NC_CACHE["nc"] = build_nc()
    return _NC_CACHE["nc"]


_PNAMES = ["in_proj_w", "conv_w", "conv_b", "x_proj_w", "dt_proj_w",
           "dt_proj_b", "A_log", "D_param", "out_proj_w"]


def kernel(**inputs):
    L, DM, DI, N, R = 1024, 1024, 2048, 16, 64
    hidden = inputs["hidden"]
    diff = inputs["diff"]
    hp = tuple(np.asarray(inputs["h_" + n]) for n in _PNAMES)
    dp = tuple(np.asarray(inputs["d_" + n]) for n in _PNAMES)

    nc = _get_nc()
    in_maps = []
    for c in range(8):
        blk, x, p = (("h", hidden, hp) if c < 4 else ("d", diff, dp))
        b = c % 4
        in_maps.append(_prep_core_inputs(np.asarray(x[b]), p, L, DM, DI, N, R))
    import time as _time
    _t0 = _time.perf_counter()
    res = run_bass_kernel_spmd(nc, in_maps, core_ids=list(range(8)))
    global LAST_RUN_SECONDS
    LAST_RUN_SECONDS = _time.perf_counter() - _t0
    outs = [np.ascontiguousarray(res.results[c]["outT"].T) for c in range(8)]
    hidden_out = np.stack(outs[0:4], axis=0).astype(np.float32)
    diff_out = np.stack(outs[4:8], axis=0).astype(np.float32)
    return (hidden_out, diff_out)


# revision 27
# speedup vs baseline: 1.9862x; 1.5440x over previous
# Bass/Trainium2 kernel for a double Mamba block (nn_ExBimamba).
#
# Sharding: 8 cores = 2 mamba blocks x 4 batch elements; each core runs the
# full per-(block,batch) computation with channels (d_inner) on SBUF
# partitions and time on the free axis. No collectives.
#
# Per-core pipeline:
#   P1 in_proj  : PE matmuls (K=d_model tiles), xz -> xin (SBUF, padded) + z (bf16 -> HBM scratch)
#   P2 conv1d   : PE diag-matmuls (4 taps, shifted moving operand) + ACT Silu(+bias)
#   P3 x_proj   : PE matmuls -> (dt|B|C); B,C broadcast to 128 partitions via HBM-bounce DMA
#   P4 scan     : per 128-ch tile g, per state n:
#                   a = ACT Exp(A[:,n] * softplus(dt_proj))   (per-partition scale)
#                   w = du16 * B_bc[n]                        (GPSIMD, bf16)
#                   h = tensor_tensor_scan(a, w)              (DVE recurrence)
#                   X = h * C_bc[n]                           (GPSIMD, bf16)
#                   y += I.T @ X                              (PE PSUM accumulate over n)
#                 then y2 = u*D + y ; y3 = y2 * silu(z)
#   P5 out_proj : PE matmuls (bf16) -> out (d_model x L), DMA out
import numpy as np
import ml_dtypes

import bass_rust
import concourse.bass as bass
import concourse.mybir as mybir
import concourse.tile as tile
from concourse.bass_utils import run_bass_kernel_spmd

F32 = mybir.dt.float32
BF16 = mybir.dt.bfloat16
AF = mybir.ActivationFunctionType
OP = mybir.AluOpType


def _split_waits(nc, max_waits=1):
    # The walrus build in this container rejects >1 sync-wait per
    # instruction; hoist extras onto preceding same-engine NoOps.
    for f in nc.m.functions:
        for bb in f.blocks:
            out = []
            for inst in bb.instructions:
                si = inst.sync_info
                if si is not None and len(si.on_wait) > max_waits:
                    waits = list(si.on_wait)
                    keep = waits[-max_waits:]
                    rest = waits[:-max_waits]
                    for i in range(0, len(rest), max_waits):
                        nop = mybir.InstNoOp(name=f"{inst.name}_ws{i}")
                        nop.engine = inst.engine
                        nop.sync_info = bass_rust.SyncInfo(
                            on_wait=rest[i : i + max_waits], on_update=[]
                        )
                        out.append(nop)
                    si.on_wait = keep
                out.append(inst)
            bb.instructions[:] = out


def build_nc(L=1024, DM=1024, DI=2048, N=16, R=64, num_devices=8, split_waits=True):
    """Build the per-core Bass program (SPMD: same program, per-core data)."""
    G = DI // 128      # d_inner tiles
    DMT = DM // 128    # d_model tiles (contraction for in_proj)
    E2 = 2 * DI // 128 # in_proj output tiles
    ET = DM // 128     # out_proj output tiles
    KH = 512           # fp32 moving free-dim max
    NH = L // KH if L >= KH else 1
    KHL = min(KH, L)

    nc = bass.Bass("TRN2", target_bir_lowering=False, debug=False,
                   num_devices=num_devices)

    # ---- external I/O (per core) ----
    xT = nc.declare_dram_parameter("xT", [DM, L], BF16, isOutput=False)
    wipT = nc.declare_dram_parameter("wipT", [DM, 2 * DI], BF16, isOutput=False)
    convw = nc.declare_dram_parameter("convw", [DI, 4], F32, isOutput=False)
    convb = nc.declare_dram_parameter("convb", [DI, 1], F32, isOutput=False)
    wxT = nc.declare_dram_parameter("wxT", [DI, R + 2 * N], BF16, isOutput=False)
    wdtT = nc.declare_dram_parameter("wdtT", [R, DI], F32, isOutput=False)
    dtb = nc.declare_dram_parameter("dtb", [DI, 1], F32, isOutput=False)
    acol = nc.declare_dram_parameter("acol", [DI, N], F32, isOutput=False)
    dcol = nc.declare_dram_parameter("dcol", [DI, 1], F32, isOutput=False)
    woutT = nc.declare_dram_parameter("woutT", [DI, DM], BF16, isOutput=False)
    eye32 = nc.declare_dram_parameter("eye32", [128, 128], F32, isOutput=False)
    eyebf = nc.declare_dram_parameter("eyebf", [128, 128], BF16, isOutput=False)
    outT = nc.declare_dram_parameter("outT", [DM, L], F32, isOutput=True)

    # ---- DRAM scratch ----
    bc_hbm = nc.dram_tensor("bc_scratch", [2 * N, L], BF16)

    from contextlib import ExitStack
    with tile.TileContext(nc) as tc:
        # persistent pools
        es0 = ExitStack()
        singles = es0.enter_context(tc.tile_pool(name="singles", bufs=1))
        u16_pool = es0.enter_context(tc.tile_pool(name="u16", bufs=1))
        bcst = es0.enter_context(tc.tile_pool(name="bcst", bufs=1))
        y3_pool = es0.enter_context(tc.tile_pool(name="y3", bufs=1))

        convw_sb = singles.tile([128, G, 4], F32)
        nc.sync.dma_start(convw_sb, convw.ap().rearrange("(g p) k -> p g k", p=128))
        convb_sb = singles.tile([128, G], F32)
        nc.sync.dma_start(convb_sb, convb.ap().rearrange("(g p) k -> p (g k)", p=128))
        dtb_sb = singles.tile([128, G], F32)
        nc.sync.dma_start(dtb_sb, dtb.ap().rearrange("(g p) k -> p (g k)", p=128))
        dcol_sb = singles.tile([128, G], F32)
        nc.sync.dma_start(dcol_sb, dcol.ap().rearrange("(g p) k -> p (g k)", p=128))
        acol_sb = singles.tile([128, G, N], F32)
        nc.sync.dma_start(acol_sb, acol.ap().rearrange("(g p) n -> p g n", p=128))
        eye32_sb = singles.tile([128, 128], F32)
        nc.sync.dma_start(eye32_sb, eye32.ap())
        eyebf_sb = singles.tile([128, 128], BF16)
        nc.sync.dma_start(eyebf_sb, eyebf.ap())

        u16_t = [u16_pool.tile([128, L], BF16, name=f"u16_{i}", tag=f"u16_{i}") for i in range(G)]
        y3_t = [y3_pool.tile([128, L], BF16, name=f"y3_{i}", tag=f"y3_{i}") for i in range(G)]

        # ---------------- P1: in_proj + P2: conv ----------------
        es1 = ExitStack()   # pools alive through P4
        xt_pool = es1.enter_context(tc.tile_pool(name="xt", bufs=1))
        wip_pool = es1.enter_context(tc.tile_pool(name="wip", bufs=4))
        p_xz = es1.enter_context(tc.tile_pool(name="p_xz", bufs=2, space="PSUM"))
        xdbl_pool = es1.enter_context(tc.tile_pool(name="xdbl", bufs=1))
        bc16_pool = es1.enter_context(tc.tile_pool(name="bc16", bufs=1))
        esA = ExitStack()   # P1/P2-only pools
        xc_pool = esA.enter_context(tc.tile_pool(name="xc", bufs=2))
        xin_pool = esA.enter_context(tc.tile_pool(name="xin", bufs=2))
        diag_pool = esA.enter_context(tc.tile_pool(name="diag", bufs=6))
        wx_pool = esA.enter_context(tc.tile_pool(name="wx", bufs=2))
        p_up = esA.enter_context(tc.tile_pool(name="p_up", bufs=1, space="PSUM"))
        p_xd = esA.enter_context(tc.tile_pool(name="p_xd", bufs=1, space="PSUM"))
        if True:

            xt_t = []
            for dm in range(DMT):
                t = xt_pool.tile([128, L], BF16, name=f"xt_{dm}", tag=f"xt_{dm}")
                nc.sync.dma_start(t, xT.ap()[dm * 128:(dm + 1) * 128, :])
                xt_t.append(t)

            F = R + 2 * N
            xd = p_xd.tile([F, L], F32)
            xin_t = []
            for e in range(G):
                ps = p_xz.tile([128, L], F32)
                for dm in range(DMT):
                    wt = wip_pool.tile([128, 128], BF16)
                    nc.sync.dma_start(
                        wt, wipT.ap()[dm * 128:(dm + 1) * 128,
                                      e * 128:(e + 1) * 128])
                    for h in range(NH):
                        nc.tensor.matmul(
                            ps[:, h * KHL:(h + 1) * KHL], wt,
                            xt_t[dm][:, h * KHL:(h + 1) * KHL],
                            start=(dm == 0), stop=(dm == DMT - 1))
                if True:
                    xi = xin_pool.tile([128, L + 4], BF16)
                    nc.vector.memset(xi[:, 0:4], 0.0)
                    nc.scalar.copy(xi[:, 4:4 + L], ps)
                    xin_t.append(xi)
                    # conv for this tile (xin slot freed right after)
                    g = e
                    up = p_up.tile([128, L], F32)
                    for k in range(4):
                        dg = diag_pool.tile([128, 128], BF16)
                        nc.vector.tensor_scalar_mul(
                            dg, eyebf_sb, convw_sb[:, g, k:k + 1])
                        for h in range(NH):
                            nc.tensor.matmul(
                                up[:, h * KHL:(h + 1) * KHL], dg,
                                xi[:, 1 + k + h * KHL:1 + k + h * KHL + KHL],
                                start=(k == 0), stop=(k == 3))
                    xc = xc_pool.tile([128, L], BF16, name=f"xc_{e}", tag="xc")
                    nc.scalar.activation(xc, up, AF.Identity,
                                         bias=convb_sb[:, g:g + 1], scale=1.0)
                    sg = xc_pool.tile([128, L], BF16, name=f"sg_{e}", tag="sg")
                    nc.scalar.activation(sg, up, AF.Sigmoid,
                                         bias=convb_sb[:, g:g + 1], scale=1.0)
                    nc.vector.tensor_mul(u16_t[g], xc, sg)
                    # x_proj contribution of this tile (PSUM accumulates over g)
                    wx = wx_pool.tile([128, F], BF16)
                    nc.sync.dma_start(wx, wxT.ap()[g * 128:(g + 1) * 128, :])
                    for h in range(NH):
                        nc.tensor.matmul(
                            xd[:, h * KHL:(h + 1) * KHL], wx,
                            u16_t[g][:, h * KHL:(h + 1) * KHL],
                            start=(g == 0), stop=(g == G - 1))

            # ---------------- P3: evict x_proj, broadcast B/C ----------------
            if True:
                xdbl_sb = xdbl_pool.tile([F, L], F32)
                nc.scalar.copy(xdbl_sb, xd)
                bc16 = bc16_pool.tile([2 * N, L], BF16)
                nc.vector.tensor_copy(bc16, xdbl_sb[R:R + 2 * N, :])
                nc.sync.dma_start(bc_hbm.ap(), bc16)

                b_bc = []
                c_bc = []
                for n in range(N):
                    bt = bcst.tile([128, L], BF16, name=f"bbc_{n}", tag=f"bbc_{n}")
                    nc.sync.dma_start(
                        bt, bc_hbm.ap()[n:n + 1, :].to_broadcast((128, L)))
                    b_bc.append(bt)
                for n in range(N):
                    ct = bcst.tile([128, L], BF16, name=f"cbc_{n}", tag=f"cbc_{n}")
                    nc.sync.dma_start(
                        ct, bc_hbm.ap()[N + n:N + n + 1, :].to_broadcast((128, L)))
                    c_bc.append(ct)

                # ---------------- P4: dt_proj + scan ----------------
                esA.close()
                wdt_pool = es1.enter_context(tc.tile_pool(name="wdt", bufs=2))
                a_pool = es1.enter_context(tc.tile_pool(name="a_sb", bufs=3))
                d_pool = es1.enter_context(tc.tile_pool(name="delta", bufs=2))
                du_pool = es1.enter_context(tc.tile_pool(name="du16", bufs=2))
                w_pool = es1.enter_context(tc.tile_pool(name="w2", bufs=3))
                h_pool = es1.enter_context(tc.tile_pool(name="h2", bufs=3))
                x_pool = es1.enter_context(tc.tile_pool(name="X2", bufs=3))
                zin_pool = es1.enter_context(tc.tile_pool(name="zin", bufs=1))
                sz_pool = es1.enter_context(tc.tile_pool(name="sz", bufs=2))
                t1_pool = es1.enter_context(tc.tile_pool(name="t1", bufs=1))
                y2_pool = es1.enter_context(tc.tile_pool(name="y2", bufs=1))
                p_a = es1.enter_context(tc.tile_pool(name="p_a", bufs=1, space="PSUM"))
                p_y = es1.enter_context(tc.tile_pool(name="p_y", bufs=1, space="PSUM"))
                if True:
                    for g in range(G):
                        # z-half in_proj for this tile, interleaved so PE has
                        # work while DVE runs the scans (z kept in SBUF).
                        zps = p_xz.tile([128, L], F32, name=f"zps_{g}", tag="ps")
                        for dm in range(DMT):
                            wt = wip_pool.tile([128, 128], BF16)
                            nc.sync.dma_start(
                                wt, wipT.ap()[dm * 128:(dm + 1) * 128,
                                              (G + g) * 128:(G + g + 1) * 128])
                            for h in range(NH):
                                nc.tensor.matmul(
                                    zps[:, h * KHL:(h + 1) * KHL], wt,
                                    xt_t[dm][:, h * KHL:(h + 1) * KHL],
                                    start=(dm == 0), stop=(dm == DMT - 1))
                        zt = zin_pool.tile([128, L], BF16)
                        nc.scalar.copy(zt, zps)

                        dtp = p_a.tile([128, L], F32, name=f"dtp_{g}", tag="dt_ps")
                        wdt = wdt_pool.tile([R, 128], F32)
                        nc.sync.dma_start(
                            wdt, wdtT.ap()[:, g * 128:(g + 1) * 128])
                        for h in range(NH):
                            nc.tensor.matmul(
                                dtp[:, h * KHL:(h + 1) * KHL], wdt,
                                xdbl_sb[0:R, h * KHL:(h + 1) * KHL],
                                start=True, stop=True)
                        edt = d_pool.tile([128, L], F32, name=f"edt_{g}", tag="edt", bufs=1)
                        nc.scalar.activation(edt, dtp, AF.Exp,
                                             bias=dtb_sb[:, g:g + 1], scale=1.0)
                        delta = d_pool.tile([128, L], BF16, name=f"delta_{g}", tag="delta")
                        nc.scalar.activation(delta, edt, AF.Ln, bias=1.0, scale=1.0)
                        du16 = du_pool.tile([128, L], BF16)
                        nc.vector.tensor_mul(du16, delta, u16_t[g])

                        y_ps = p_y.tile([128, L], F32)
                        for n in range(N):
                            a = a_pool.tile([128, L], BF16, name=f"a_{g}_{n}", tag="a_sb")
                            nc.scalar.activation(a, delta, AF.Exp,
                                                 scale=acol_sb[:, g, n:n + 1])
                            w2 = w_pool.tile([128, L], BF16)
                            weng = nc.gpsimd if (n % 2 == 0) else nc.vector
                            weng.tensor_mul(w2, du16, b_bc[n])
                            h2 = h_pool.tile([128, L], BF16)
                            nc.vector.tensor_tensor_scan(
                                h2, a, w2, 0.0, op0=OP.mult, op1=OP.add)
                            X2 = x_pool.tile([128, L], BF16)
                            xeng = nc.gpsimd if (n % 3 == 0) else nc.vector
                            xeng.tensor_mul(X2, h2, c_bc[n])
                            for h in range(NH):
                                nc.tensor.matmul(
                                    y_ps[:, h * KHL:(h + 1) * KHL], eyebf_sb,
                                    X2[:, h * KHL:(h + 1) * KHL],
                                    start=(n == 0), stop=(n == N - 1))
                        t1 = t1_pool.tile([128, L], BF16)
                        nc.vector.tensor_scalar_mul(t1, u16_t[g],
                                                    dcol_sb[:, g:g + 1])
                        y2 = y2_pool.tile([128, L], BF16)
                        nc.vector.tensor_add(y2, t1, y_ps)
                        sz = sz_pool.tile([128, L], BF16)
                        nc.scalar.activation(sz, zt, AF.Sigmoid)
                        y3a = sz_pool.tile([128, L], BF16, name=f"y3a_{g}", tag="y3a")
                        nc.gpsimd.tensor_mul(y3a, y2, zt)
                        nc.vector.tensor_mul(y3_t[g], y3a, sz)

        # ---------------- P5: out_proj ----------------
        es1.close()
        es5 = ExitStack()
        wo_pool = es5.enter_context(tc.tile_pool(name="wo", bufs=4))
        osb_pool = es5.enter_context(tc.tile_pool(name="osb", bufs=3))
        p_out = es5.enter_context(tc.tile_pool(name="p_out", bufs=3, space="PSUM"))
        if True:
            for e in range(ET):
                ps = p_out.tile([128, L], F32)
                for g in range(G):
                    wo = wo_pool.tile([128, 128], BF16)
                    nc.sync.dma_start(
                        wo, woutT.ap()[g * 128:(g + 1) * 128,
                                       e * 128:(e + 1) * 128])
                    for h in range(NH):
                        nc.tensor.matmul(
                            ps[:, h * KHL:(h + 1) * KHL], wo,
                            y3_t[g][:, h * KHL:(h + 1) * KHL],
                            start=(g == 0), stop=(g == G - 1))
                osb = osb_pool.tile([128, L], F32)
                nc.scalar.copy(osb, ps)
                nc.sync.dma_start(outT.ap()[e * 128:(e + 1) * 128, :], osb)

        es5.close()
        es0.close()

    if split_waits:
        _split_waits(nc)
    return nc


def _prep_core_inputs(x_b, p, L, DM, DI, N, R):
    """Host-side packing for one core. p = tuple of 9 block params."""
    (in_proj_w, conv_w, conv_b, x_proj_w, dt_proj_w, dt_proj_b,
     A_log, D_param, out_proj_w) = p
    bf = ml_dtypes.bfloat16
    f32 = np.float32
    return {
        "xT": np.ascontiguousarray(x_b.T.astype(np.float32)).astype(bf),
        "wipT": np.ascontiguousarray(in_proj_w.T.astype(np.float32)).astype(bf),
        "convw": np.ascontiguousarray(conv_w, dtype=f32),
        "convb": np.ascontiguousarray(conv_b.reshape(DI, 1), dtype=f32),
        "wxT": np.ascontiguousarray(x_proj_w.T.astype(np.float32)).astype(bf),
        "wdtT": np.ascontiguousarray(dt_proj_w.T, dtype=f32),
        "dtb": np.ascontiguousarray(dt_proj_b.reshape(DI, 1), dtype=f32),
        "acol": np.ascontiguousarray(-np.exp(A_log), dtype=f32),
        "dcol": np.ascontiguousarray(D_param.reshape(DI, 1), dtype=f32),
        "woutT": np.ascontiguousarray(out_proj_w.T).astype(bf),
        "eye32": np.eye(128, dtype=f32),
        "eyebf": np.eye(128).astype(bf),
    }


LAST_RUN_SECONDS = None
_NC_CACHE = {}


def _get_nc():
    if "nc" not in _NC_CACHE:
        _NC_CACHE["nc"] = build_nc()
    return _NC_CACHE["nc"]


_PNAMES = ["in_proj_w", "conv_w", "conv_b", "x_proj_w", "dt_proj_w",
           "dt_proj_b", "A_log", "D_param", "out_proj_w"]


def kernel(**inputs):
    L, DM, DI, N, R = 1024, 1024, 2048, 16, 64
    hidden = inputs["hidden"]
    diff = inputs["diff"]
    hp = tuple(np.asarray(inputs["h_" + n]) for n in _PNAMES)
    dp = tuple(np.asarray(inputs["d_" + n]) for n in _PNAMES)

    nc = _get_nc()
    in_maps = []
    for c in range(8):
        blk, x, p = (("h", hidden, hp) if c < 4 else ("d", diff, dp))
        b = c % 4
        in_maps.append(_prep_core_inputs(np.asarray(x[b]), p, L, DM, DI, N, R))
    import time as _time
    _t0 = _time.perf_counter()
    res = run_bass_kernel_spmd(nc, in_maps, core_ids=list(range(8)))
    global LAST_RUN_SECONDS
    LAST_RUN_SECONDS = _time.perf_counter() - _t0
    outs = [np.ascontiguousarray(res.results[c]["outT"].T) for c in range(8)]
    hidden_out = np.stack(outs[0:4], axis=0).astype(np.float32)
    diff_out = np.stack(outs[4:8], axis=0).astype(np.float32)
    return (hidden_out, diff_out)
